# revision 8
# baseline (speedup 1.0000x reference)
"""Conv2d(128->256, 3x3, pad=1) over (32,128,56,56), data-parallel across 8
NeuronCores (4 images per core).

Per core: conv = 9 shifted accumulating matmuls per output tile.
  - contraction K = Cin = 128 (partition dim)
  - stationary lhsT = W^T[ci, co_tile] per (ky,kx)  -> [128, 128] bf16
  - moving rhs = input pixels [128, <=8 rows, <=56 cols] (N <= 448)
  - PSUM accumulates the 9 (ky,kx) taps; padding handled by clipping each
    tap's matmul to the valid rectangle (center tap goes first with
    start=True and covers the full tile).
Bias is added during the PSUM->SBUF copy (VectorE tensor_scalar), writing
bf16 (host converts the gathered output back to f32).

Schedule (from trace analysis of the 110.9us baseline):
  - mid-stream MMs pace at N/2.4GHz + ~5ns already (roofline); the fat is
    the entry ramp (first data MM at 11.1us) and the tail (5.3us after the
    last MM).
  - rings are FIFO at ~175GB/s with ~1.5-2.3us DGE first-packet latency and
    DMA instrs can only issue after engine boot (~6.5us).  Entry plan:
      sync:   w cot0 (ready ~9.7) -> bias -> w cot1 -> x2 full
      scalar: x0 rows in tapered pieces 10/8/16/14/8 (first ready ~9.7,
              then always ahead of the ~8rows/0.85us compute cadence)
              -> x1 full -> x3 full
    gpsimd (slow SWDGE queue) carries nothing.
  - PE prewarm: a CONTIGUOUS ~3.4us bridge of dummy matmuls (8 x N=512,
    back-to-back cold gap is N/1.2 ns) on a VectorE-memset tile, ending at
    data-ready, so the HAM clock gate is warm when the first data chunk
    issues.  (Reading an unwritten tile is rejected by Tile, and a shorter
    bridge leaves a PE idle gap that stops the HAM busy-window from ever
    firing -- both measured dead ends.)
  - tail: the last (img,cot) block tapers its chunks to 8x5,6,2 rows with
    stores (0,14)(14,28)(28,44)(44,54)(54,56) on alternating rings, so the
    exit barrier's trailing DMA is a 2-row (28KB) transfer instead of the
    baseline's 4-row store behind a 20-row one.
"""

import numpy as np
import ml_dtypes

import concourse.mybir as mybir
import concourse.tile as tile
from concourse import bacc
from concourse.bass_utils import run_bass_kernel_spmd

N_CORES = 8
B, CIN, H, W = 32, 128, 56, 56
COUT, R, S = 256, 3, 3
BL = B // N_CORES          # images per core
NCOT = COUT // 128         # Cout tiles of 128
YCHUNK = 8                 # output rows per matmul tile
NYC = H // YCHUNK

MM_DT = mybir.dt.bfloat16
MM_NP = ml_dtypes.bfloat16

NWARM = 8                  # dummy matmuls bridging entry barrier -> first data
WARM_N = 512               # free dim of each dummy matmul; the bridge must be
                           # CONTIGUOUS ~3.4us of PE busy (back-to-back cold gap
                           # is N/1.2 ns) ending at data-ready (~10.5us), else
                           # the HAM clock gate never warms and the data stream
                           # runs at 1.2GHz for microseconds
X0_SPLITS = [(0, 10), (10, 18), (18, 34), (34, 48), (48, 56)]
OUT_SPLITS = {1: (0, 14), 3: (14, 28), 5: (28, 48), 6: (48, 56)}  # yc -> store rows
# tap order in the weight layout: center tap first (it is the start=True
# matmul that covers the full PSUM tile)
TAP_ORDER = [(1, 1), (0, 0), (0, 1), (0, 2), (1, 0), (1, 2), (2, 0), (2, 1), (2, 2)]

_cache = {}


def _build():
    if "nc" in _cache:
        return _cache["nc"]
    nc = bacc.Bacc("TRN2", target_bir_lowering=False, debug=False)
    f32 = mybir.dt.float32
    x_d = nc.dram_tensor("x", [BL, CIN, H, W], MM_DT, kind="ExternalInput").ap()
    w_d = nc.dram_tensor("w", [CIN, NCOT, R * S, 128], MM_DT, kind="ExternalInput").ap()
    b_d = nc.dram_tensor("b", [128, NCOT], f32, kind="ExternalInput").ap()
    y_d = nc.dram_tensor("y", [BL, COUT, H, W], MM_DT, kind="ExternalOutput").ap()

    with tile.TileContext(nc) as tc:
        with (
            tc.tile_pool(name="consts", bufs=1) as cpool,
            tc.tile_pool(name="xin", bufs=BL) as xpool,
            tc.tile_pool(name="yout", bufs=3) as opool,
            tc.tile_pool(name="ps", bufs=8, space="PSUM") as pspool,
        ):
            # --- PE prewarm: the warm tile is tiny so the VectorE memset it
            # depends on retires ~0.4us earlier than a full-width one ---
            warm_x = cpool.tile([128, WARM_N], MM_DT)
            nc.vector.memset(warm_x[:], 0.0)
            warm_ps = pspool.tile([128, WARM_N], f32, tag="ps")
            for _ in range(NWARM):
                nc.tensor.matmul(
                    warm_ps[:], warm_x[:, 0:128], warm_x[:], start=True, stop=True
                )

            # --- loads, in intended per-ring FIFO order ---
            w_sb = cpool.tile([CIN, NCOT, R * S, 128], MM_DT)
            b_sb = cpool.tile([128, NCOT], f32)
            x_tiles = [
                xpool.tile([CIN, H, W], MM_DT, name=f"x_sb_{img}", tag="x_sb")
                for img in range(BL)
            ]
            # sync ring: critical weights, bias, then bulk
            nc.sync.dma_start(w_sb[:, 0], w_d[:, 0])
            nc.sync.dma_start(b_sb[:], b_d[:])
            nc.sync.dma_start(w_sb[:, 1], w_d[:, 1])
            nc.sync.dma_start(x_tiles[2][:], x_d[2])
            # scalar ring: x0 prefix pieces sized to the compute cadence
            for r0, r1 in X0_SPLITS:
                nc.scalar.dma_start(x_tiles[0][:, r0:r1, :], x_d[0, :, r0:r1, :])
            nc.scalar.dma_start(x_tiles[1][:], x_d[1])
            nc.scalar.dma_start(x_tiles[3][:], x_d[3])

            store_rings = [nc.sync, nc.scalar]
            store_cnt = 0

            norm_chunks = [(YCHUNK * yc, YCHUNK) for yc in range(NYC)]
            norm_stores = dict(OUT_SPLITS)
            # last block: taper to a 2-row final chunk so the exit barrier
            # only waits on a tiny trailing transfer
            last_chunks = norm_chunks[:-1] + [(48, 6), (54, 2)]
            last_stores = {
                1: (0, 14), 3: (14, 28), 5: (28, 44), 6: (44, 54), 7: (54, 56)
            }

            for img in range(BL):
                x_sb = x_tiles[img]
                for cot in range(NCOT):
                    last_block = img == BL - 1 and cot == NCOT - 1
                    chunks = last_chunks if last_block else norm_chunks
                    stores = last_stores if last_block else norm_stores
                    o_sb = opool.tile(
                        [128, H, W], MM_DT, name=f"o_sb_{img}_{cot}", tag="o_sb"
                    )
                    for yc, (y0, rows) in enumerate(chunks):
                        ps = pspool.tile(
                            [128, rows, W], f32, name=f"ps_{img}_{cot}_{yc}", tag="ps"
                        )
                        # center tap first: full-tile write with start=True
                        nc.tensor.matmul(
                            ps[:],
                            w_sb[:, cot, 0, :],
                            x_sb[:, y0 : y0 + rows, :],
                            start=True,
                            stop=False,
                        )
                        for ti, (ky, kx) in enumerate(TAP_ORDER[1:], start=1):
                            oy0 = max(0, 1 - ky - y0)
                            oy1 = min(rows, H + 1 - y0 - ky)
                            ox0 = max(0, 1 - kx)
                            ox1 = min(W, W + 1 - kx)
                            nc.tensor.matmul(
                                ps[:, oy0:oy1, ox0:ox1],
                                w_sb[:, cot, ti, :],
                                x_sb[
                                    :,
                                    y0 + oy0 + ky - 1 : y0 + oy1 + ky - 1,
                                    ox0 + kx - 1 : ox1 + kx - 1,
                                ],
                                start=False,
                                stop=(ti == R * S - 1),
                            )
                        # PSUM -> SBUF with fused bias add, all on VectorE
                        # (no ACTIVATE => Scalar never loads its LUT)
                        nc.vector.tensor_scalar_add(
                            o_sb[:, y0 : y0 + rows],
                            ps[:],
                            b_sb[:, cot : cot + 1],
                        )
                        # store finished row bands, alternating fast rings
                        if yc in stores:
                            r0, r1 = stores[yc]
                            eng = store_rings[store_cnt % 2]
                            store_cnt += 1
                            eng.dma_start(
                                y_d[img, 128 * cot : 128 * (cot + 1), r0:r1, :],
                                o_sb[:, r0:r1, :],
                            )

    nc.compile()
    _cache["nc"] = nc
    return nc


def _in_maps(inputs, weight, bias):
    x = np.asarray(inputs).astype(MM_NP)
    # weight (co, ci, ky, kx) -> (ci, cot, tap, co_in_tile), taps in TAP_ORDER
    wt = (
        np.asarray(weight)
        .reshape(NCOT, 128, CIN, R, S)
        .transpose(2, 0, 3, 4, 1)  # (ci, cot, ky, kx, co)
        .astype(MM_NP)
    )
    w = np.ascontiguousarray(
        np.stack([wt[:, :, ky, kx, :] for ky, kx in TAP_ORDER], axis=2)
    )
    b = np.ascontiguousarray(
        np.asarray(bias).astype(np.float32).reshape(NCOT, 128).T
    )
    return [
        {"x": np.ascontiguousarray(x[c * BL : (c + 1) * BL]), "w": w, "b": b}
        for c in range(N_CORES)
    ]


def kernel(inputs, weight, bias):
    nc = _build()
    in_maps = _in_maps(inputs, weight, bias)
    res = run_bass_kernel_spmd(nc, in_maps, core_ids=list(range(N_CORES)))
    out = np.concatenate([res.results[c]["y"] for c in range(N_CORES)], axis=0)
    return out.astype(np.float32)


# revision 19
# speedup vs baseline: 1.0174x; 1.0174x over previous
"""Conv2d(128->256, 3x3, pad=1) over (32,128,56,56), data-parallel across 8
NeuronCores (4 images per core).

Per core: conv = 9 shifted accumulating matmuls per output tile.
  - contraction K = Cin = 128 (partition dim)
  - stationary lhsT = W^T[ci, co_tile] per (ky,kx)  -> [128, 128] bf16
  - moving rhs = input pixels [128, <=8 rows, <=56 cols] (N <= 448)
  - PSUM accumulates the 9 (ky,kx) taps; padding handled by clipping each
    tap's matmul to the valid rectangle (center tap goes first with
    start=True and covers the full tile).
Bias is added during the PSUM->SBUF copy (VectorE tensor_scalar), writing
bf16 (host converts the gathered output back to f32).

Schedule (from trace analysis; mid-stream MMs already pace at N/2.4GHz +
~5ns = roofline, so all the recoverable time is at the entry and the tail):
  - Entry: rings are FIFO; DMA instrs can only issue after engine boot.
    sync (arms fastest) carries the x0 prefix in six pieces whose
    boundaries exactly match the interleaved phase-1 row reads (first
    usable boot+~1.7us) then images 1-2; scalar carries w cot0 in three
    3-tap pieces (first usable boot+~1.8us) then bias, w cot1, x3; gpsimd
    (slow SWDGE) carries nothing.  Real data matmuls start at boot+~1.8us.
  - A dummy-matmul bridge (NWARM x N=512 on a VectorE-memset tile; reading
    an unwritten tile is rejected by Tile) keeps the PE busy from engine
    boot to data-ready with NO idle gap: a PE idle gap while the HAM clock
    gate is cold delays the 2.4GHz un-throttle by ~3.4us (measured).
  - The first (img,cot) block interleaves chunks 0-3 across 4 PSUM banks in
    two phases (phase 1 yc-outer over taps {0,1,2} following the x stream;
    phase 2 ti-outer over taps 3-8 following the w pieces), so neither
    input stream can starve the PE while still cold.  Early chunks run at
    the cold 1.2GHz rate until the HAM fires (~boot+3.4) -- real work at
    half rate beats idling.
  - Tail: the last block tapers its chunks to 8x5,6,2 rows with stores
    (0,14)(14,28)(28,44)(44,54)(54,56) on alternating rings, so the exit
    barrier's trailing DMA is a 2-row (28KB) transfer.
"""

import numpy as np
import ml_dtypes

import concourse.mybir as mybir
import concourse.tile as tile
from concourse import bacc
from concourse.bass_utils import run_bass_kernel_spmd

N_CORES = 8
B, CIN, H, W = 32, 128, 56, 56
COUT, R, S = 256, 3, 3
BL = B // N_CORES          # images per core
NCOT = COUT // 128         # Cout tiles of 128
YCHUNK = 8                 # output rows per matmul tile
NYC = H // YCHUNK

MM_DT = mybir.dt.bfloat16
MM_NP = ml_dtypes.bfloat16

NWARM = 5                  # dummy matmuls bridging entry barrier -> first data:
                           # measured ~353ns each, so 5 end ~boot+1.77us, right
                           # at measured data-ready -- the PE busy period stays
                           # CONTIGUOUS (v11's 3-dummy bridge left a ~0.75us gap
                           # at boot+1.1 that can delay the HAM un-throttle);
                           # the data stream then runs cold until the busy
                           # window fires (~boot+3.4) -- better than idling
WARM_N = 512               # free dim of each dummy matmul (cold gap = N/1.2 ns)
# piece boundaries at 16/24/32 match the interleaved phase-1 row reads
# exactly (chunk yc's taps {0,1,2} read rows <= 8*yc+7), so every phase-1
# chunk has positive arrival margin (the 18/26/34 boundaries left chunk 1
# waiting ~0.4us on rows it never reads)
X0_SPLITS = [(0, 10), (10, 16), (16, 24), (24, 32), (32, 44), (44, 56)]
OUT_SPLITS = {1: (0, 14), 3: (14, 28), 5: (28, 48), 6: (48, 56)}  # yc -> store rows
# tap order in the weight layout: center tap first (it is the start=True
# matmul that covers the full PSUM tile)
TAP_ORDER = [(1, 1), (0, 0), (0, 1), (0, 2), (1, 0), (1, 2), (2, 0), (2, 1), (2, 2)]

_cache = {}


def _build():
    if "nc" in _cache:
        return _cache["nc"]
    nc = bacc.Bacc("TRN2", target_bir_lowering=False, debug=False)
    f32 = mybir.dt.float32
    x_d = nc.dram_tensor("x", [BL, CIN, H, W], MM_DT, kind="ExternalInput").ap()
    w_d = nc.dram_tensor("w", [CIN, NCOT, R * S, 128], MM_DT, kind="ExternalInput").ap()
    b_d = nc.dram_tensor("b", [128, NCOT], f32, kind="ExternalInput").ap()
    y_d = nc.dram_tensor("y", [BL, COUT, H, W], MM_DT, kind="ExternalOutput").ap()

    with tile.TileContext(nc) as tc:
        with (
            tc.tile_pool(name="consts", bufs=1) as cpool,
            tc.tile_pool(name="xin", bufs=BL) as xpool,
            tc.tile_pool(name="yout", bufs=3) as opool,
            tc.tile_pool(name="ps", bufs=8, space="PSUM") as pspool,
        ):
            # --- PE prewarm: contiguous dummy-matmul bridge (see NWARM) ---
            warm_x = cpool.tile([128, WARM_N], MM_DT)
            nc.vector.memset(warm_x[:], 0.0)
            warm_ps = pspool.tile([128, WARM_N], f32, tag="ps")
            for _ in range(NWARM):
                nc.tensor.matmul(
                    warm_ps[:], warm_x[:, 0:128], warm_x[:], start=True, stop=True
                )

            # --- loads, in intended per-ring FIFO order ---
            w_sb = cpool.tile([CIN, NCOT, R * S, 128], MM_DT)
            b_sb = cpool.tile([128, NCOT], f32)
            x_tiles = [
                xpool.tile([CIN, H, W], MM_DT, name=f"x_sb_{img}", tag="x_sb")
                for img in range(BL)
            ]
            # sync ring arms fastest: it carries the x0 prefix pieces (the
            # tightest gate, first piece usable boot+~1.7us), then images 1-2.
            # scalar carries w cot0 in 3-tap pieces (first piece usable
            # boot+~1.8us so the interleaved first block can start immediately),
            # then bias, w cot1, x3.
            for r0, r1 in X0_SPLITS:
                nc.sync.dma_start(x_tiles[0][:, r0:r1, :], x_d[0, :, r0:r1, :])
            for t0, t1 in [(0, 3), (3, 6), (6, 9)]:
                nc.scalar.dma_start(w_sb[:, 0, t0:t1], w_d[:, 0, t0:t1])
            nc.sync.dma_start(x_tiles[1][:], x_d[1])
            nc.scalar.dma_start(b_sb[:], b_d[:])
            nc.scalar.dma_start(w_sb[:, 1], w_d[:, 1])
            nc.sync.dma_start(x_tiles[2][:], x_d[2])
            nc.scalar.dma_start(x_tiles[3][:], x_d[3])

            store_rings = [nc.sync, nc.scalar]
            store_cnt = 0

            norm_chunks = [(YCHUNK * yc, YCHUNK) for yc in range(NYC)]
            norm_stores = dict(OUT_SPLITS)
            # last block: taper to a 2-row final chunk so the exit barrier
            # only waits on a tiny trailing transfer
            last_chunks = norm_chunks[:-1] + [(48, 6), (54, 2)]
            last_stores = {
                1: (0, 14), 3: (14, 28), 5: (28, 44), 6: (44, 54), 7: (54, 56)
            }

            def mm_tap(ps, x_sb, cot, y0, rows, ti, start, stop):
                ky, kx = TAP_ORDER[ti]
                oy0 = max(0, 1 - ky - y0)
                oy1 = min(rows, H + 1 - y0 - ky)
                ox0 = max(0, 1 - kx)
                ox1 = min(W, W + 1 - kx)
                nc.tensor.matmul(
                    ps[:, oy0:oy1, ox0:ox1],
                    w_sb[:, cot, ti, :],
                    x_sb[
                        :,
                        y0 + oy0 + ky - 1 : y0 + oy1 + ky - 1,
                        ox0 + kx - 1 : ox1 + kx - 1,
                    ],
                    start=start,
                    stop=stop,
                )

            def drain_and_store(o_sb, ps, img, cot, yc, y0, rows, stores):
                nonlocal store_cnt
                # PSUM -> SBUF with fused bias add, all on VectorE
                # (no ACTIVATE => Scalar never loads its LUT)
                nc.vector.tensor_scalar_add(
                    o_sb[:, y0 : y0 + rows],
                    ps[:],
                    b_sb[:, cot : cot + 1],
                )
                # store finished row bands, alternating fast rings
                if yc in stores:
                    r0, r1 = stores[yc]
                    eng = store_rings[store_cnt % 2]
                    store_cnt += 1
                    eng.dma_start(
                        y_d[img, 128 * cot : 128 * (cot + 1), r0:r1, :],
                        o_sb[:, r0:r1, :],
                    )

            for img in range(BL):
                x_sb = x_tiles[img]
                for cot in range(NCOT):
                    first_block = img == 0 and cot == 0
                    last_block = img == BL - 1 and cot == NCOT - 1
                    chunks = last_chunks if last_block else norm_chunks
                    stores = last_stores if last_block else norm_stores
                    o_sb = opool.tile(
                        [128, H, W], MM_DT, name=f"o_sb_{img}_{cot}", tag="o_sb"
                    )
                    start_yc = 0
                    if first_block:
                        # Interleave chunks 0-3 across 4 PSUM banks so the PE
                        # (starting on real data at boot+~2.2us, well before the
                        # full w cot0 lands) never idles: an idle gap while the
                        # HAM is cold can stop the busy-window from firing and
                        # costs ~2.5us.  Phase 1 walks chunks with taps {0,1,2}
                        # (yc-outer, following the x row stream on sync); phase
                        # 2 walks taps 3-8 across all four chunks (ti-outer,
                        # following the 3-tap w pieces on scalar).  Tap 0 of
                        # each chunk is the full-coverage center tap
                        # (start=True).
                        start_yc = 4
                        ps_i = [
                            pspool.tile([128, YCHUNK, W], f32,
                                        name=f"ps_{img}_{cot}_{yc}", tag="ps")
                            for yc in range(4)
                        ]
                        for yc in range(4):
                            for ti in range(3):
                                mm_tap(ps_i[yc], x_sb, cot, YCHUNK * yc, YCHUNK,
                                       ti, start=(ti == 0), stop=False)
                        for ti in range(3, R * S):
                            for yc in range(4):
                                mm_tap(ps_i[yc], x_sb, cot, YCHUNK * yc, YCHUNK,
                                       ti, start=False, stop=(ti == R * S - 1))
                        for yc in range(4):
                            drain_and_store(o_sb, ps_i[yc], img, cot, yc,
                                            YCHUNK * yc, YCHUNK, stores)
                    for yc, (y0, rows) in enumerate(chunks):
                        if yc < start_yc:
                            continue
                        ps = pspool.tile(
                            [128, rows, W], f32, name=f"ps_{img}_{cot}_{yc}", tag="ps"
                        )
                        for ti in range(R * S):
                            mm_tap(ps, x_sb, cot, y0, rows, ti,
                                   start=(ti == 0), stop=(ti == R * S - 1))
                        drain_and_store(o_sb, ps, img, cot, yc, y0, rows, stores)

    nc.compile()
    _cache["nc"] = nc
    return nc


def _in_maps(inputs, weight, bias):
    x = np.asarray(inputs).astype(MM_NP)
    # weight (co, ci, ky, kx) -> (ci, cot, tap, co_in_tile), taps in TAP_ORDER
    wt = (
        np.asarray(weight)
        .reshape(NCOT, 128, CIN, R, S)
        .transpose(2, 0, 3, 4, 1)  # (ci, cot, ky, kx, co)
        .astype(MM_NP)
    )
    w = np.ascontiguousarray(
        np.stack([wt[:, :, ky, kx, :] for ky, kx in TAP_ORDER], axis=2)
    )
    b = np.ascontiguousarray(
        np.asarray(bias).astype(np.float32).reshape(NCOT, 128).T
    )
    return [
        {"x": np.ascontiguousarray(x[c * BL : (c + 1) * BL]), "w": w, "b": b}
        for c in range(N_CORES)
    ]


def kernel(inputs, weight, bias):
    nc = _build()
    in_maps = _in_maps(inputs, weight, bias)
    res = run_bass_kernel_spmd(nc, in_maps, core_ids=list(range(N_CORES)))
    out = np.concatenate([res.results[c]["y"] for c in range(N_CORES)], axis=0)
    return out.astype(np.float32)
